# revision 3
# baseline (speedup 1.0000x reference)
"""MoE gate kernel for TRN2 (8 NeuronCores, Bass/Tile).

Computes, for hidden_states [8, 8192, 512] f32 and gate weight [4, 512] f32:
  logits = x @ W^T, scores = softmax(logits), top-2 (values normalized) and
  the seq-aux load-balancing loss — matching the reference MoEGate module.

Sharding: data-parallel over the batch dim — core c handles batch row c
(8192 tokens). The tiny weight is replicated. The scalar aux_loss partials
([scores_colsum, expert_counts] per core) are combined on the host.

Per-core dataflow, token t = p*64 + i (p = SBUF partition, i = tile 0..63):
  phase 1 (per 128-token tile): DMA x in 1 MiB chunks -> PE transpose chunks
  [128t,128h] -> [128h,128t] (fp32 transpose is bit-exact) -> 4 accumulating
  fp32 matmuls against W^T chunks -> logits [128,4] in PSUM -> batched copy
  into an SBUF plane buffer logits_all [128, 4*64] (plane e at cols 64e+i).
  phase 2 (all tokens at once, [128,64] plane ops): top-2 via min/max
  identities, argmax via is_equal masks, weights via exp + reciprocal,
  softmax column-sums and expert counts reduced per partition, then one
  ones-vector matmul to reduce across partitions.
"""

import numpy as np

import concourse.bass as bass
import concourse.tile as tile
from concourse import mybir
from concourse.bass_utils import run_bass_kernel_spmd

N_CORES = 8
BSZ, SEQ, H = 8, 8192, 512
E = 4  # experts
K = 2  # top-k
ALPHA = 0.1
T = SEQ  # tokens per core
P = 128  # partitions
NT = T // P  # 64 token-tiles per core
DMA_GROUP = 4  # token-tiles per input DMA (1 MiB)
LG_BATCH = 16  # token-tiles per PSUM logits batch

F32 = mybir.dt.float32
I32 = mybir.dt.int32
Alu = mybir.AluOpType
Act = mybir.ActivationFunctionType


def _split_excess_waits(nc):
    """walrus in this container allows 1 sync wait per instruction (2 for
    EventSemaphore); Tile's final drain can carry more. Move extras to NOPs."""
    caps = {"InstEventSemaphore": 2}
    for bbname, bb in nc.bb_map.items():
        insts = list(bb.bb.instructions)
        out, changed = [], False
        for i in insts:
            si = i.sync_info
            cap = caps.get(type(i).__name__, 1)
            if si is not None and len(si.on_wait) > cap:
                waits = list(si.on_wait)
                for k, w in enumerate(waits[cap:]):
                    nop = mybir.InstNoOp(name=f"{i.name}-ws{k}", engine=i.engine)
                    nop.sync_info = mybir.SyncInfo(on_wait=[w], on_update=[])
                    nc.register_instruction(nop)
                    out.append(nop)
                i.sync_info = mybir.SyncInfo(
                    on_wait=waits[:cap], on_update=list(si.on_update)
                )
                changed = True
            out.append(i)
        if changed:
            bb.bb.instructions = out


def build():
    nc = bass.Bass("TRN2", target_bir_lowering=False, debug=False, num_devices=1)
    x = nc.dram_tensor("x", [T, H], F32, kind="ExternalInput")
    w = nc.dram_tensor("w", [E, H], F32, kind="ExternalInput")
    ident = nc.dram_tensor("ident", [P, P], F32, kind="ExternalInput")
    ones = nc.dram_tensor("ones", [P, 1], F32, kind="ExternalInput")
    o_idx = nc.dram_tensor("o_idx", [T, K], I32, kind="ExternalOutput")
    o_wgt = nc.dram_tensor("o_wgt", [T, K], F32, kind="ExternalOutput")
    o_aux = nc.dram_tensor("o_aux", [1, 2 * E], F32, kind="ExternalOutput")

    NCH = H // P  # 4 h-chunks
    x_dram = x.ap().rearrange("(p i) h -> p i h", p=P)  # [128, 64, 512]

    with tile.TileContext(nc) as tc:
        with (
            tc.tile_pool(name="consts", bufs=1) as consts,
            tc.tile_pool(name="xin", bufs=3) as xin,
            tc.tile_pool(name="xtp", bufs=3, space="PSUM") as xtp,
            tc.tile_pool(name="xts", bufs=3) as xts,
            tc.tile_pool(name="lgp", bufs=2, space="PSUM") as lgp,
            tc.tile_pool(name="auxp", bufs=1, space="PSUM") as auxp,
            tc.tile_pool(name="ph2", bufs=1) as ph2,
        ):
            id_sb = consts.tile([P, P], F32)
            nc.sync.dma_start(id_sb[:], ident[:])
            ones_sb = consts.tile([P, 1], F32)
            nc.sync.dma_start(ones_sb[:], ones[:])
            # W^T chunk c ([128 h, 4 e]) lives at cols 4c..4c+3
            wt_sb = consts.tile([P, E * NCH], F32)
            wr = w.ap().rearrange("e h -> h e")
            for c in range(NCH):
                nc.sync.dma_start(
                    wt_sb[:, E * c : E * (c + 1)], wr[P * c : P * (c + 1), :]
                )

            # logits planes: plane e at cols [64e, 64e+64)
            logits_all = ph2.tile([P, E * NT], F32)

            lg_p = None
            for k in range(NT // DMA_GROUP):
                x_ld = xin.tile([P, DMA_GROUP * H], F32)
                nc.sync.dma_start(
                    x_ld[:].rearrange("p (i h) -> p i h", i=DMA_GROUP),
                    x_dram[:, DMA_GROUP * k : DMA_GROUP * (k + 1), :],
                )
                for j in range(DMA_GROUP):
                    i = DMA_GROUP * k + j
                    xs = x_ld[:, H * j : H * (j + 1)]
                    xt_p = xtp.tile([P, H], F32)
                    for c in range(NCH):
                        nc.tensor.matmul(
                            xt_p[:, P * c : P * (c + 1)],
                            xs[:, P * c : P * (c + 1)],
                            id_sb[:],
                            is_transpose=True,
                            start=(c == 0),
                            stop=(c == NCH - 1),
                        )
                    xt_s = xts.tile([P, H], F32)
                    nc.vector.tensor_copy(xt_s[:, 0 : H // 2], xt_p[:, 0 : H // 2])
                    nc.scalar.copy(xt_s[:, H // 2 : H], xt_p[:, H // 2 : H])

                    m = i % LG_BATCH
                    if m == 0:
                        lg_p = lgp.tile([P, E * LG_BATCH], F32)
                    for c in range(NCH):
                        nc.tensor.matmul(
                            lg_p[:, E * m : E * (m + 1)],
                            xt_s[:, P * c : P * (c + 1)],
                            wt_sb[:, E * c : E * (c + 1)],
                            start=(c == 0),
                            stop=(c == NCH - 1),
                        )
                    if m == LG_BATCH - 1:
                        b = i // LG_BATCH
                        # [p, (t e)] -> planes: out col = 64e + 16b + t
                        src = lg_p[:].rearrange("p (t e) -> p e t", e=E)
                        dst = logits_all[:].rearrange("p (e t) -> p e t", e=E)[
                            :, :, LG_BATCH * b : LG_BATCH * (b + 1)
                        ]
                        eng = nc.vector if (b % 2 == 0) else nc.scalar
                        if eng is nc.vector:
                            nc.vector.tensor_copy(dst, src)
                        else:
                            nc.scalar.copy(dst, src)

            # ---------------- phase 2 ----------------
            def pl(e):
                return logits_all[:, NT * e : NT * (e + 1)]

            A, B, C, D = pl(0), pl(1), pl(2), pl(3)
            S = [P, NT]
            mx_ab = ph2.tile(S, F32)
            nc.vector.tensor_tensor(mx_ab[:], A, B, Alu.max)
            mx_cd = ph2.tile(S, F32)
            nc.vector.tensor_tensor(mx_cd[:], C, D, Alu.max)
            mn_ab = ph2.tile(S, F32)
            nc.vector.tensor_tensor(mn_ab[:], A, B, Alu.min)
            mn_cd = ph2.tile(S, F32)
            nc.vector.tensor_tensor(mn_cd[:], C, D, Alu.min)
            m1 = ph2.tile(S, F32)
            nc.vector.tensor_tensor(m1[:], mx_ab[:], mx_cd[:], Alu.max)
            t1 = ph2.tile(S, F32)
            nc.vector.tensor_tensor(t1[:], mx_ab[:], mx_cd[:], Alu.min)
            t2 = ph2.tile(S, F32)
            nc.vector.tensor_tensor(t2[:], mn_ab[:], mn_cd[:], Alu.max)
            m2 = ph2.tile(S, F32)
            nc.vector.tensor_tensor(m2[:], t1[:], t2[:], Alu.max)

            eq1 = [ph2.tile(S, F32, name=f"eq1_{e}") for e in range(E)]
            eq2 = [ph2.tile(S, F32, name=f"eq2_{e}") for e in range(E)]
            for e in range(E):
                nc.vector.tensor_tensor(eq1[e][:], pl(e), m1[:], Alu.is_equal)
                nc.vector.tensor_tensor(eq2[e][:], pl(e), m2[:], Alu.is_equal)

            # idx = 1*eqB + 2*eqC + 3*eqD  (float, exact)
            idx0 = ph2.tile(S, F32)
            idx1 = ph2.tile(S, F32)
            tmp = ph2.tile(S, F32)
            nc.vector.scalar_tensor_tensor(
                tmp[:], eq1[2][:], 2.0, eq1[1][:], Alu.mult, Alu.add
            )
            nc.vector.scalar_tensor_tensor(
                idx0[:], eq1[3][:], 3.0, tmp[:], Alu.mult, Alu.add
            )
            tmp2 = ph2.tile(S, F32)
            nc.vector.scalar_tensor_tensor(
                tmp2[:], eq2[2][:], 2.0, eq2[1][:], Alu.mult, Alu.add
            )
            nc.vector.scalar_tensor_tensor(
                idx1[:], eq2[3][:], 3.0, tmp2[:], Alu.mult, Alu.add
            )

            # weights: w0 = 1/(1+exp(m2-m1)), w1 = exp(m2-m1)*w0
            d21 = ph2.tile(S, F32)
            nc.vector.tensor_tensor(d21[:], m2[:], m1[:], Alu.subtract)
            e2 = ph2.tile(S, F32)
            nc.scalar.activation(e2[:], d21[:], Act.Exp)
            den = ph2.tile(S, F32)
            nc.vector.tensor_scalar_add(den[:], e2[:], 1.0)
            w0 = ph2.tile(S, F32)
            nc.vector.reciprocal(w0[:], den[:])
            w1 = ph2.tile(S, F32)
            nc.vector.tensor_tensor(w1[:], e2[:], w0[:], Alu.mult)

            # softmax scores (for aux): exp(l - m1) planes, Z, col-sums
            expb = ph2.tile([P, E * NT], F32)
            for e in range(E):
                nc.vector.tensor_tensor(
                    expb[:, NT * e : NT * (e + 1)], pl(e), m1[:], Alu.subtract
                )
            nc.scalar.activation(expb[:], expb[:], Act.Exp)

            def ex(e):
                return expb[:, NT * e : NT * (e + 1)]

            z01 = ph2.tile(S, F32)
            nc.vector.tensor_tensor(z01[:], ex(0), ex(1), Alu.add)
            z23 = ph2.tile(S, F32)
            nc.vector.tensor_tensor(z23[:], ex(2), ex(3), Alu.add)
            zz = ph2.tile(S, F32)
            nc.vector.tensor_tensor(zz[:], z01[:], z23[:], Alu.add)
            rz = ph2.tile(S, F32)
            nc.vector.reciprocal(rz[:], zz[:])

            aux_sb = ph2.tile([P, 2 * E], F32)
            junk = ph2.tile(S, F32)
            for e in range(E):
                # sum_t softmax[t, e]: out = (exp*1.0) * rz, accum = sum(out)
                nc.vector.scalar_tensor_tensor(
                    junk[:],
                    ex(e),
                    1.0,
                    rz[:],
                    Alu.mult,
                    Alu.mult,
                    accum_out=aux_sb[:, e : e + 1],
                )
                # count of topk hits per expert
                nc.vector.scalar_tensor_tensor(
                    junk[:],
                    eq1[e][:],
                    1.0,
                    eq2[e][:],
                    Alu.mult,
                    Alu.add,
                    accum_out=aux_sb[:, E + e : E + e + 1],
                )

            aux_ps = auxp.tile([1, 2 * E], F32)
            nc.tensor.matmul(
                aux_ps[:], ones_sb[:], aux_sb[:], start=True, stop=True
            )
            aux_out = ph2.tile([1, 2 * E], F32)
            nc.vector.tensor_copy(aux_out[:], aux_ps[:])
            nc.sync.dma_start(o_aux[:], aux_out[:])

            # interleave outputs and store
            out_idx = ph2.tile([P, NT * K], I32)
            out_wgt = ph2.tile([P, NT * K], F32)
            oi = out_idx[:].rearrange("p (t k) -> p t k", k=K)
            ow = out_wgt[:].rearrange("p (t k) -> p t k", k=K)
            nc.vector.tensor_copy(oi[:, :, 0], idx0[:])
            nc.vector.tensor_copy(oi[:, :, 1], idx1[:])
            nc.vector.tensor_copy(ow[:, :, 0], w0[:])
            nc.vector.tensor_copy(ow[:, :, 1], w1[:])
            nc.sync.dma_start(
                o_idx.ap().rearrange("(p t) k -> p t k", p=P),
                out_idx[:].rearrange("p (t k) -> p t k", k=K),
            )
            nc.sync.dma_start(
                o_wgt.ap().rearrange("(p t) k -> p t k", p=P),
                out_wgt[:].rearrange("p (t k) -> p t k", k=K),
            )
    _split_excess_waits(nc)
    return nc


_NC_CACHE = None


def _get_nc():
    global _NC_CACHE
    if _NC_CACHE is None:
        _NC_CACHE = build()
    return _NC_CACHE


def _in_maps(hidden_states, weight):
    ident = np.eye(P, dtype=np.float32)
    ones = np.ones((P, 1), dtype=np.float32)
    maps = []
    for c in range(N_CORES):
        # token t of core c = batch row c, seq p*64+i
        maps.append(
            {
                "x": np.ascontiguousarray(hidden_states[c]),
                "w": np.ascontiguousarray(weight),
                "ident": ident,
                "ones": ones,
            }
        )
    return maps


def _combine(results):
    idx = np.concatenate([r["o_idx"] for r in results], axis=0)
    wgt = np.concatenate([r["o_wgt"] for r in results], axis=0)
    aux_parts = np.stack([r["o_aux"][0] for r in results], axis=0)  # [8, 8]
    scores_sum = aux_parts[:, :E].astype(np.float64)  # [8, 4]
    counts = aux_parts[:, E:].astype(np.float64)  # [8, 4]
    ce = counts / (SEQ * K / E)
    pi = scores_sum / SEQ
    aux = np.float32(ALPHA * np.mean(np.sum(ce * pi, axis=1)))
    return idx.astype(np.int32), wgt.astype(np.float32), np.asarray(aux, np.float32)


def run(hidden_states, weight, trace=False, **spmd_kwargs):
    nc = _get_nc()
    res = run_bass_kernel_spmd(
        nc,
        _in_maps(hidden_states, weight),
        core_ids=list(range(N_CORES)),
        trace=trace,
        **spmd_kwargs,
    )
    out = _combine(res.results)
    return out, res


def kernel(hidden_states, weight):
    out, _ = run(hidden_states, weight)
    return out


# revision 5
# speedup vs baseline: 1.1195x; 1.1195x over previous
"""MoE gate kernel for TRN2 (8 NeuronCores, Bass/Tile).

Computes, for hidden_states [8, 8192, 512] f32 and gate weight [4, 512] f32:
  logits = x @ W^T, scores = softmax(logits), top-2 (values normalized) and
  the seq-aux load-balancing loss — matching the reference MoEGate module.

Sharding: data-parallel over the batch dim — core c handles batch row c
(8192 tokens). The tiny weight is replicated. The scalar aux_loss partials
([scores_colsum, expert_counts] per core) are combined on the host.

Per-core dataflow, token t = p*64 + i (p = SBUF partition, i = tile 0..63):
  phase 1 (per 128-token tile): DMA x in 1 MiB chunks -> PE transpose chunks
  [128t,128h] -> [128h,128t] (fp32 transpose is bit-exact) -> 4 accumulating
  fp32 matmuls against W^T chunks -> logits [128,4] in PSUM -> batched copy
  into an SBUF plane buffer logits_all [128, 4*64] (plane e at cols 64e+i).
  phase 2 (all tokens at once, [128,64] plane ops): top-2 via min/max
  identities, argmax via is_equal masks, weights via exp + reciprocal,
  softmax column-sums and expert counts reduced per partition, then one
  ones-vector matmul to reduce across partitions.
"""

import numpy as np

import concourse.bass as bass
import concourse.tile as tile
from concourse import mybir
from concourse.bass_utils import run_bass_kernel_spmd

N_CORES = 8
BSZ, SEQ, H = 8, 8192, 512
E = 4  # experts
K = 2  # top-k
ALPHA = 0.1
T = SEQ  # tokens per core
P = 128  # partitions
NT = T // P  # 64 token-tiles per core
DMA_GROUP = 4  # token-tiles per input DMA (1 MiB)
LG_BATCH = 16  # token-tiles per PSUM logits batch

F32 = mybir.dt.float32
I32 = mybir.dt.int32
Alu = mybir.AluOpType
Act = mybir.ActivationFunctionType


def _split_excess_waits(nc):
    """walrus in this container allows 1 sync wait per instruction (2 for
    EventSemaphore); Tile's final drain can carry more. Move extras to NOPs."""
    caps = {"InstEventSemaphore": 2}
    for bbname, bb in nc.bb_map.items():
        insts = list(bb.bb.instructions)
        out, changed = [], False
        for i in insts:
            si = i.sync_info
            cap = caps.get(type(i).__name__, 1)
            if si is not None and len(si.on_wait) > cap:
                waits = list(si.on_wait)
                for k, w in enumerate(waits[cap:]):
                    nop = mybir.InstNoOp(name=f"{i.name}-ws{k}", engine=i.engine)
                    nop.sync_info = mybir.SyncInfo(on_wait=[w], on_update=[])
                    nc.register_instruction(nop)
                    out.append(nop)
                i.sync_info = mybir.SyncInfo(
                    on_wait=waits[:cap], on_update=list(si.on_update)
                )
                changed = True
            out.append(i)
        if changed:
            bb.bb.instructions = out


def build():
    nc = bass.Bass("TRN2", target_bir_lowering=False, debug=False, num_devices=1)
    x = nc.dram_tensor("x", [T, H], F32, kind="ExternalInput")
    w = nc.dram_tensor("w", [E, H], F32, kind="ExternalInput")
    ident = nc.dram_tensor("ident", [P, P], F32, kind="ExternalInput")
    ones = nc.dram_tensor("ones", [P, 1], F32, kind="ExternalInput")
    o_idx = nc.dram_tensor("o_idx", [T, K], I32, kind="ExternalOutput")
    o_wgt = nc.dram_tensor("o_wgt", [T, K], F32, kind="ExternalOutput")
    o_aux = nc.dram_tensor("o_aux", [1, 2 * E], F32, kind="ExternalOutput")

    NCH = H // P  # 4 h-chunks
    GT = 4  # token-tiles per matmul group (N = GT*P = 512 moving cols)
    NG = NT // GT  # 16 groups
    x_dram = x.ap().rearrange("(p i) h -> p i h", p=P)  # [128, 64, 512]

    with tile.TileContext(nc) as tc:
        with (
            tc.tile_pool(name="consts", bufs=1) as consts,
            tc.tile_pool(name="xin", bufs=3) as xin,
            tc.tile_pool(name="xtp", bufs=3, space="PSUM") as xtp,
            tc.tile_pool(name="xtg", bufs=2) as xtg,
            tc.tile_pool(name="lgtp", bufs=2, space="PSUM") as lgtp,
            tc.tile_pool(name="lgts", bufs=2) as lgts,
            tc.tile_pool(name="lgtok", bufs=2, space="PSUM") as lgtokp,
            tc.tile_pool(name="auxp", bufs=1, space="PSUM") as auxp,
            tc.tile_pool(name="ph2", bufs=1) as ph2,
        ):
            id_sb = consts.tile([P, P], F32)
            nc.sync.dma_start(id_sb[:], ident[:])
            ones_sb = consts.tile([P, 1], F32)
            nc.sync.dma_start(ones_sb[:], ones[:])
            # W^T chunk c ([128 h, 4 e]) lives at cols 4c..4c+3
            wt_sb = consts.tile([P, E * NCH], F32)
            wr = w.ap().rearrange("e h -> h e")
            for c in range(NCH):
                nc.sync.dma_start(
                    wt_sb[:, E * c : E * (c + 1)], wr[P * c : P * (c + 1), :]
                )

            # logits planes: plane e at cols [64e, 64e+64)
            logits_all = ph2.tile([P, E * NT], F32)

            # HAM warm-up: dense junk matmuls while the first x DMA streams.
            # They only read the (tiny, early) weight/identity tiles.
            warm_ps = lgtokp.tile([P, E * LG_BATCH], F32, name="warm", tag="lgtok")
            for r in range(20):
                nc.tensor.matmul(
                    warm_ps[0:E, :],
                    wt_sb[:, 0:E],
                    id_sb[:, 0 : E * LG_BATCH],
                    start=(r == 0),
                    stop=(r == 19),
                )

            lgtok_ps = None
            pending_bt = []  # deferred back-transposes: (group, lgT_sb tile)

            def emit_bt(g, lgt_s):
                nonlocal lgtok_ps
                for j in range(GT):
                    i = GT * g + j  # token-tile index
                    m = i % LG_BATCH
                    if m == 0:
                        lgtok_ps = lgtokp.tile([P, E * LG_BATCH], F32, name="lgtok_ps", tag="lgtok")
                    nc.tensor.matmul(
                        lgtok_ps[:, E * m : E * (m + 1)],
                        lgt_s[0:E, P * j : P * (j + 1)],
                        id_sb[0:E, 0:E],
                        is_transpose=True,
                        start=True,
                        stop=True,
                    )
                    if m == LG_BATCH - 1:
                        b = i // LG_BATCH
                        src = lgtok_ps[:].rearrange("p (t e) -> p e t", e=E)
                        dst = logits_all[:].rearrange("p (e t) -> p e t", e=E)[
                            :, :, LG_BATCH * b : LG_BATCH * (b + 1)
                        ]
                        if b % 2 == 0:
                            nc.vector.tensor_copy(dst, src)
                        else:
                            nc.scalar.copy(dst, src)

            for g in range(NG):
                x_ld = xin.tile([P, GT * H], F32)
                nc.sync.dma_start(
                    x_ld[:].rearrange("p (i h) -> p i h", i=GT),
                    x_dram[:, GT * g : GT * (g + 1), :],
                )
                # chunk-major xT for this group: block (c, j) at cols 512c+128j
                xt_g = xtg.tile([P, NCH * GT * P], F32)
                xt_g3 = xt_g[:].rearrange("p (c t) -> p c t", c=NCH)
                for j in range(GT):
                    xt_p = xtp.tile([P, H], F32)
                    for c in range(NCH):
                        nc.tensor.matmul(
                            xt_p[:, P * c : P * (c + 1)],
                            x_ld[:, H * j + P * c : H * j + P * (c + 1)],
                            id_sb[:],
                            is_transpose=True,
                            start=(c == 0),
                            stop=(c == NCH - 1),
                        )
                    xt_p3 = xt_p[:].rearrange("p (c t) -> p c t", c=NCH)
                    nc.vector.tensor_copy(
                        xt_g3[:, 0:2, P * j : P * (j + 1)], xt_p3[:, 0:2, :]
                    )
                    nc.scalar.copy(
                        xt_g3[:, 2:4, P * j : P * (j + 1)], xt_p3[:, 2:4, :]
                    )

                # back-transposes of the PREVIOUS group sit here in PE order so
                # they never stall on the lgT copy
                if pending_bt:
                    emit_bt(*pending_bt.pop())

                lgt_p = lgtp.tile([E, GT * P], F32)
                for c in range(NCH):
                    nc.tensor.matmul(
                        lgt_p[:],
                        wt_sb[:, E * c : E * (c + 1)],
                        xt_g[:, GT * P * c : GT * P * (c + 1)],
                        start=(c == 0),
                        stop=(c == NCH - 1),
                    )
                lgt_s = lgts.tile([E, GT * P], F32)
                if g % 2 == 0:
                    nc.vector.tensor_copy(lgt_s[:], lgt_p[:])
                else:
                    nc.scalar.copy(lgt_s[:], lgt_p[:])
                pending_bt.append((g, lgt_s))

            while pending_bt:
                emit_bt(*pending_bt.pop())

            # ---------------- phase 2 ----------------
            def pl(e):
                return logits_all[:, NT * e : NT * (e + 1)]

            A, B, C, D = pl(0), pl(1), pl(2), pl(3)
            S = [P, NT]
            mx_ab = ph2.tile(S, F32)
            nc.vector.tensor_tensor(mx_ab[:], A, B, Alu.max)
            mx_cd = ph2.tile(S, F32)
            nc.vector.tensor_tensor(mx_cd[:], C, D, Alu.max)
            mn_ab = ph2.tile(S, F32)
            nc.vector.tensor_tensor(mn_ab[:], A, B, Alu.min)
            mn_cd = ph2.tile(S, F32)
            nc.vector.tensor_tensor(mn_cd[:], C, D, Alu.min)
            m1 = ph2.tile(S, F32)
            nc.vector.tensor_tensor(m1[:], mx_ab[:], mx_cd[:], Alu.max)
            t1 = ph2.tile(S, F32)
            nc.vector.tensor_tensor(t1[:], mx_ab[:], mx_cd[:], Alu.min)
            t2 = ph2.tile(S, F32)
            nc.vector.tensor_tensor(t2[:], mn_ab[:], mn_cd[:], Alu.max)
            m2 = ph2.tile(S, F32)
            nc.vector.tensor_tensor(m2[:], t1[:], t2[:], Alu.max)

            eq1 = [ph2.tile(S, F32, name=f"eq1_{e}") for e in range(E)]
            eq2 = [ph2.tile(S, F32, name=f"eq2_{e}") for e in range(E)]
            for e in range(E):
                nc.vector.tensor_tensor(eq1[e][:], pl(e), m1[:], Alu.is_equal)
                nc.vector.tensor_tensor(eq2[e][:], pl(e), m2[:], Alu.is_equal)

            # idx = 1*eqB + 2*eqC + 3*eqD  (float, exact)
            idx0 = ph2.tile(S, F32)
            idx1 = ph2.tile(S, F32)
            tmp = ph2.tile(S, F32)
            nc.vector.scalar_tensor_tensor(
                tmp[:], eq1[2][:], 2.0, eq1[1][:], Alu.mult, Alu.add
            )
            nc.vector.scalar_tensor_tensor(
                idx0[:], eq1[3][:], 3.0, tmp[:], Alu.mult, Alu.add
            )
            tmp2 = ph2.tile(S, F32)
            nc.vector.scalar_tensor_tensor(
                tmp2[:], eq2[2][:], 2.0, eq2[1][:], Alu.mult, Alu.add
            )
            nc.vector.scalar_tensor_tensor(
                idx1[:], eq2[3][:], 3.0, tmp2[:], Alu.mult, Alu.add
            )

            # weights: w0 = 1/(1+exp(m2-m1)), w1 = exp(m2-m1)*w0
            d21 = ph2.tile(S, F32)
            nc.vector.tensor_tensor(d21[:], m2[:], m1[:], Alu.subtract)
            e2 = ph2.tile(S, F32)
            nc.scalar.activation(e2[:], d21[:], Act.Exp)
            den = ph2.tile(S, F32)
            nc.vector.tensor_scalar_add(den[:], e2[:], 1.0)
            w0 = ph2.tile(S, F32)
            nc.vector.reciprocal(w0[:], den[:])
            w1 = ph2.tile(S, F32)
            nc.vector.tensor_tensor(w1[:], e2[:], w0[:], Alu.mult)

            # softmax scores (for aux): exp(l - m1) planes, Z, col-sums
            expb = ph2.tile([P, E * NT], F32)
            for e in range(E):
                nc.vector.tensor_tensor(
                    expb[:, NT * e : NT * (e + 1)], pl(e), m1[:], Alu.subtract
                )
            nc.scalar.activation(expb[:], expb[:], Act.Exp)

            def ex(e):
                return expb[:, NT * e : NT * (e + 1)]

            z01 = ph2.tile(S, F32)
            nc.vector.tensor_tensor(z01[:], ex(0), ex(1), Alu.add)
            z23 = ph2.tile(S, F32)
            nc.vector.tensor_tensor(z23[:], ex(2), ex(3), Alu.add)
            zz = ph2.tile(S, F32)
            nc.vector.tensor_tensor(zz[:], z01[:], z23[:], Alu.add)
            rz = ph2.tile(S, F32)
            nc.vector.reciprocal(rz[:], zz[:])

            aux_sb = ph2.tile([P, 2 * E], F32)
            junk = ph2.tile(S, F32)
            for e in range(E):
                # sum_t softmax[t, e]: out = (exp*1.0) * rz, accum = sum(out)
                nc.vector.scalar_tensor_tensor(
                    junk[:],
                    ex(e),
                    1.0,
                    rz[:],
                    Alu.mult,
                    Alu.mult,
                    accum_out=aux_sb[:, e : e + 1],
                )
                # count of topk hits per expert
                nc.vector.scalar_tensor_tensor(
                    junk[:],
                    eq1[e][:],
                    1.0,
                    eq2[e][:],
                    Alu.mult,
                    Alu.add,
                    accum_out=aux_sb[:, E + e : E + e + 1],
                )

            aux_ps = auxp.tile([1, 2 * E], F32)
            nc.tensor.matmul(
                aux_ps[:], ones_sb[:], aux_sb[:], start=True, stop=True
            )
            aux_out = ph2.tile([1, 2 * E], F32)
            nc.vector.tensor_copy(aux_out[:], aux_ps[:])
            nc.sync.dma_start(o_aux[:], aux_out[:])

            # interleave outputs and store
            out_idx = ph2.tile([P, NT * K], I32)
            out_wgt = ph2.tile([P, NT * K], F32)
            oi = out_idx[:].rearrange("p (t k) -> p t k", k=K)
            ow = out_wgt[:].rearrange("p (t k) -> p t k", k=K)
            nc.vector.tensor_copy(oi[:, :, 0], idx0[:])
            nc.vector.tensor_copy(oi[:, :, 1], idx1[:])
            nc.vector.tensor_copy(ow[:, :, 0], w0[:])
            nc.vector.tensor_copy(ow[:, :, 1], w1[:])
            nc.sync.dma_start(
                o_idx.ap().rearrange("(p t) k -> p t k", p=P),
                out_idx[:].rearrange("p (t k) -> p t k", k=K),
            )
            nc.sync.dma_start(
                o_wgt.ap().rearrange("(p t) k -> p t k", p=P),
                out_wgt[:].rearrange("p (t k) -> p t k", k=K),
            )
    _split_excess_waits(nc)
    return nc


_NC_CACHE = None


def _get_nc():
    global _NC_CACHE
    if _NC_CACHE is None:
        _NC_CACHE = build()
    return _NC_CACHE


def _in_maps(hidden_states, weight):
    ident = np.eye(P, dtype=np.float32)
    ones = np.ones((P, 1), dtype=np.float32)
    maps = []
    for c in range(N_CORES):
        # token t of core c = batch row c, seq p*64+i
        maps.append(
            {
                "x": np.ascontiguousarray(hidden_states[c]),
                "w": np.ascontiguousarray(weight),
                "ident": ident,
                "ones": ones,
            }
        )
    return maps


def _combine(results):
    idx = np.concatenate([r["o_idx"] for r in results], axis=0)
    wgt = np.concatenate([r["o_wgt"] for r in results], axis=0)
    aux_parts = np.stack([r["o_aux"][0] for r in results], axis=0)  # [8, 8]
    scores_sum = aux_parts[:, :E].astype(np.float64)  # [8, 4]
    counts = aux_parts[:, E:].astype(np.float64)  # [8, 4]
    ce = counts / (SEQ * K / E)
    pi = scores_sum / SEQ
    aux = np.float32(ALPHA * np.mean(np.sum(ce * pi, axis=1)))
    return idx.astype(np.int32), wgt.astype(np.float32), np.asarray(aux, np.float32)


def run(hidden_states, weight, trace=False, **spmd_kwargs):
    nc = _get_nc()
    res = run_bass_kernel_spmd(
        nc,
        _in_maps(hidden_states, weight),
        core_ids=list(range(N_CORES)),
        trace=trace,
        **spmd_kwargs,
    )
    out = _combine(res.results)
    return out, res


def kernel(hidden_states, weight):
    out, _ = run(hidden_states, weight)
    return out
